# revision 18
# baseline (speedup 1.0000x reference)
"""Single-head attention (B=4, N=2048, D=1024), scores scaled by 10.

Sharding: 8 cores = (batch, query-half). Core 2b+h owns queries
[1024h:1024(h+1)] of batch b. There is NO cross-core communication.

Algebraic restructuring (both sides of the softmax):
  scores = Q K^T = x (Wq^T Wk) x^T       -> A = Wq^T Wk on host,
                                             XA = x_own @ A on device,
                                             keys = raw full x
  out    = P V   = (P x_full) Wv^T       -> PX on device (PV-shaped),
                                             then one projection by Wv
The K and V projections and both pair-exchanges disappear; total
matmul work is unchanged (XA + QK + PX + PO = old Q/K/V + QK + PV) but
every collective, DRAM round-trip, and pair-skew dependency is gone.

Numerics: single-pass fp16 matmuls (fp32 PSUM accumulate); measured
end-to-end rel err ~4.6e-3 vs the fp32 reference (2e-2 gate). The
per-query max subtraction cancels exactly in the softmax
normalization, so max precision only guards overflow.

Schedule: inputs host-pre-tiled into contiguous >=1MB per-use DMAs
split across the two HWDGE rings; phase A is just XA (~28us); phase B
is a 2-deep software pipeline per 256-query chunk:
QK -> max-tree (interleaved with PSUM copies) -> exp -> sums -> PX ->
PO, with the next chunks' QK matmuls emitted around each softmax so
the PE never waits on the DVE/ACT chain.
"""

import numpy as np

B, SEQ, D = 4, 2048, 1024
NQ = 1024          # queries per core
QCH = 512          # attention q-chunk
NCH = NQ // QCH
NCORES = 8
DT = D // 128      # 8 d-tiles
ET = D // 128      # 8 e-tiles
KT = SEQ // 128    # 16 k-tiles

_BUILT = {}


def _build():
    if "nc" in _BUILT:
        return _BUILT["nc"]
    from contextlib import ExitStack

    import concourse.bass as bass  # noqa: F401
    import concourse.mybir as mybir
    import concourse.tile as tile
    from concourse import bacc

    dt = mybir.dt
    F32, F16 = dt.float32, dt.float16
    AL = mybir.AluOpType
    EXP = mybir.ActivationFunctionType.Exp

    nc = bacc.Bacc("TRN2", target_bir_lowering=False, debug=False)

    # host-pre-tiled inputs: each leading index is one contiguous DMA
    xq_d = nc.dram_tensor("xq", [8, 128, 2, 512], F16, kind="ExternalInput")
    xk_d = nc.dram_tensor("xk", [4, 128, DT, 512], F16, kind="ExternalInput")
    xr_d = nc.dram_tensor("xr", [4, 128, 4, D], F16, kind="ExternalInput")
    am_d = nc.dram_tensor("am", [ET, 128, DT, 128], F16, kind="ExternalInput")
    wv_d = nc.dram_tensor("wv", [ET, 128, DT, 128], F16, kind="ExternalInput")
    ot_d = nc.dram_tensor("ot", [D, NQ], F16, kind="ExternalOutput")

    xq_r = xq_d.ap()
    xk_r = xk_d.ap()
    xr_r = xr_d.ap()
    am_r = am_d.ap()
    wv_r = wv_d.ap()
    ot_r = ot_d.ap().rearrange("(t p) q -> p t q", p=128)

    with tile.TileContext(nc) as tc, ExitStack() as ctx:
        big_pool = ctx.enter_context(tc.tile_pool(name="big", bufs=1))
        # XA^T [d-part, d-tile, own queries]
        xat = big_pool.tile([128, ET, NQ], F16, tag="xat")
        # full x^T (the key matrix), 4 column blocks of 512 keys
        xk_t = [
            big_pool.tile([128, DT, 512], F16, tag=f"xk{i}", name=f"xk{i}")
            for i in range(4)
        ]
        # full x row-tiled (for PX), 4 blocks of 4 k-tiles
        xr_t = [
            big_pool.tile([128, 4, D], F16, tag=f"xr{i}", name=f"xr{i}")
            for i in range(4)
        ]
        # Wv^T tiled for the output projection
        wv_t = [
            big_pool.tile([128, DT, 128], F16, tag=f"wv{e}", name=f"wv{e}")
            for e in range(ET)
        ]

        const_pool = ctx.enter_context(tc.tile_pool(name="const", bufs=1))
        ones16 = const_pool.tile([128, 1], F16, tag="ones16")
        ten16 = const_pool.tile([1, 128], F16, tag="ten16")
        one32 = const_pool.tile([1, 128], F32, tag="one32")
        nc.vector.memset(ones16[:], 1.0)
        nc.vector.memset(ten16[:], 10.0)
        nc.vector.memset(one32[:], 1.0)

        with (
            tc.tile_pool(name="xspan", bufs=1) as xspan,
            tc.tile_pool(name="wstr", bufs=1) as wpool,
            tc.tile_pool(name="psA", bufs=4, space="PSUM") as psA,
        ):
            xq_t = [
                xspan.tile([128, 2, 512], F16, tag=f"xq{i}", name=f"xq{i}")
                for i in range(8)
            ]
            am_t = [
                wpool.tile([128, DT, 128], F16, tag=f"am{e}", name=f"am{e}")
                for e in range(ET)
            ]
            # input DMAs up front, split across the two HWDGE rings (sync ->
            # qSPDynamicHW, scalar -> qActDynamicHW), ordered by first use
            nc.sync.dma_start(xq_t[0][:], xq_r[0])
            nc.sync.dma_start(am_t[0][:], am_r[0])
            for i in range(1, 8):
                nc.sync.dma_start(xq_t[i][:], xq_r[i])
            for e in range(1, ET):
                nc.sync.dma_start(am_t[e][:], am_r[e])
            for i in range(4):
                nc.sync.dma_start(xk_t[i][:], xk_r[i])
            for i in range(4):
                nc.scalar.dma_start(xr_t[i][:], xr_r[i])
            for e in range(ET):
                nc.scalar.dma_start(wv_t[e][:], wv_r[e])

            # ------------- Phase A: XA^T = A^T @ x_own^T -------------------
            for et in range(ET):
                for chn in range(2):
                    n0 = 512 * chn
                    ps = psA.tile([128, 512], F32, tag="psA")
                    for dti in range(DT):
                        nc.tensor.matmul(
                            ps[:],
                            am_t[et][:, dti, :],
                            xq_t[4 * chn + dti // 2][:, dti % 2, :],
                            start=(dti == 0),
                            stop=(dti == DT - 1),
                        )
                    nc.vector.tensor_copy(xat[:, et, n0 : n0 + 512], ps[:])

        # ---------------- Phase B: attention, q-chunked, pipelined --------
        with (
            tc.tile_pool(name="stp", bufs=2) as stpool,
            tc.tile_pool(name="pp", bufs=1) as ppool,
            tc.tile_pool(name="pxp", bufs=1) as pxpool,
            tc.tile_pool(name="runm", bufs=2) as runpool,
            tc.tile_pool(name="tree", bufs=1) as treepool,
            tc.tile_pool(name="aux", bufs=1) as auxpool,
            tc.tile_pool(name="osb", bufs=2) as outpool,
            tc.tile_pool(name="psS", bufs=2, space="PSUM") as psS,
            tc.tile_pool(name="psPX", bufs=2, space="PSUM") as psPX,
            tc.tile_pool(name="psPO", bufs=2, space="PSUM") as psPO,
            tc.tile_pool(name="psX", bufs=1, space="PSUM") as psX,
            tc.tile_pool(name="psR", bufs=1, space="PSUM") as psR,
        ):
            st_ap = {}
            t8_ap = {}
            p_ap = {}
            maxb_ap = {}
            m1_ap = {}

            def emit_qk_half(c, half, piggy=None):
                """Scores for k-tiles [8h, 8h+8) of chunk c, with the DVE
                max-tree levels interleaved behind the PSUM copies. `piggy`
                is a list of thunks (previous chunk's stt/exp) drained a few
                per k-tile so they don't clog the DVE ahead of the copies."""
                q0 = QCH * c
                if half == 0:
                    st_ap[c] = stpool.tile([128, KT, QCH], F32, tag="st", name="st")
                    t8_ap[c] = runpool.tile([128, QCH], F32, tag="rm", name="rm")
                st, rm = st_ap[c], t8_ap[c]
                for kti in range(8 * half, 8 * half + 8):
                    k0 = 128 * (kti % 4)
                    ps = psS.tile([128, QCH], F32, tag="psS")
                    for dti in range(DT):
                        nc.tensor.matmul(
                            ps[:],
                            xk_t[kti // 4][:, dti, k0 : k0 + 128],
                            xat[:, dti, q0 : q0 + QCH],
                            start=(dti == 0),
                            stop=(dti == DT - 1),
                        )
                    nc.vector.tensor_copy(st[:, kti, :], ps[:])
                    # running per-partition max, paced behind the copies
                    if kti == 0:
                        nc.vector.tensor_copy(rm[:], st[:, 0, :])
                    else:
                        nc.vector.tensor_max(rm[:], rm[:], st[:, kti, :])
                    if piggy:
                        for _ in range(min(3, len(piggy))):
                            piggy.pop(0)()
                while piggy:
                    piggy.pop(0)()

            def emit_tree_finish(c):
                """Final fold 128 partitions -> one [1, QCH] max row."""
                rm = t8_ap[c]
                rm16 = treepool.tile([128, QCH], F16, tag="rm16")
                nc.vector.tensor_copy(rm16[:], rm[:])
                fold4 = treepool.tile([32, 4, QCH], F16, tag="fold4")
                for a in range(4):
                    nc.sync.dma_start(
                        fold4[:, a, :], rm16[32 * a : 32 * (a + 1), :]
                    )
                nc.vector.tensor_max(fold4[:, 0, :], fold4[:, 0, :], fold4[:, 1, :])
                nc.vector.tensor_max(fold4[:, 2, :], fold4[:, 2, :], fold4[:, 3, :])
                nc.vector.tensor_max(fold4[:, 0, :], fold4[:, 0, :], fold4[:, 2, :])
                t32t = treepool.tile([32, QCH], F16, tag="t32t")
                nc.vector.transpose(t32t[:], fold4[:, 0, :])
                mx32 = treepool.tile([32, 32], F16, tag="mx32")
                nc.vector.memset(mx32[:], 0.0)
                nc.vector.reduce_max(
                    mx32[:, 0 : QCH // 32],
                    t32t[:].rearrange("p (j c) -> p j c", c=32),
                    axis=mybir.AxisListType.X,
                )
                mx32t = treepool.tile([32, 32], F16, tag="mx32t")
                nc.vector.transpose(mx32t[:], mx32[:])
                m1row = treepool.tile([1, QCH], F16, tag="m1row")
                nc.sync.dma_start(m1row[:], mx32t[0 : QCH // 32, :])
                m1_ap[c] = m1row

            def emit_maxb(c):
                """Broadcast 10*max across partitions via rank-1 matmul."""
                maxb_ps = psX.tile([128, QCH], F32, tag="bcast")
                nc.tensor.matmul(
                    maxb_ps[:], ten16[:], m1_ap[c][:], start=True, stop=True
                )
                maxb = auxpool.tile([128, QCH], F32, tag="maxb")
                nc.vector.tensor_copy(maxb[:], maxb_ps[:])
                maxb_ap[c] = maxb

            def emit_stt(c):
                """st = 10*st - maxb (in place, DVE)."""
                st, maxb = st_ap[c], maxb_ap[c]
                for kti in range(KT):
                    nc.vector.scalar_tensor_tensor(
                        st[:, kti, :],
                        st[:, kti, :],
                        10.0,
                        maxb[:],
                        op0=AL.mult,
                        op1=AL.subtract,
                    )

            def emit_exp(c):
                """P = exp(st) in fp16 (batched ACT)."""
                st = st_ap[c]
                p_ap[c] = ppool.tile([128, KT, QCH], F16, tag="p", name="p")
                for kti in range(3, KT, 4):
                    nc.scalar.activation(
                        p_ap[c][:, kti - 3 : kti + 1, :],
                        st[:, kti - 3 : kti + 1, :],
                        EXP,
                    )

            def emit_sum_recb(c):
                """Key-sums of P via ones-matmul, 1/sum, broadcast."""
                sum_ps = psR.tile([1, QCH], F32, tag="sum")
                for kti in range(KT):
                    nc.tensor.matmul(
                        sum_ps[:],
                        ones16[:],
                        p_ap[c][:, kti, :],
                        start=(kti == 0),
                        stop=(kti == KT - 1),
                    )
                recrow = treepool.tile([1, QCH], F32, tag="recrow")
                nc.vector.reciprocal(recrow[:], sum_ps[:])
                recb_ps = psX.tile([128, QCH], F32, tag="bcast")
                nc.tensor.matmul(
                    recb_ps[:], one32[:], recrow[:], start=True, stop=True
                )
                recb = auxpool.tile([128, QCH], F32, tag="recb")
                nc.vector.tensor_copy(recb[:], recb_ps[:])
                return recb

            def emit_px_po(c, recb, piggy=None):
                """PX^T = x^T P^T, then O^T = Wv PX^T scaled by 1/sum.
                `piggy` thunks (next chunk's stt) drain behind the copies."""
                q0 = QCH * c
                pxt = pxpool.tile([128, DT, QCH], F16, tag="pxt", name="pxt")
                for dti in range(DT):
                    d0 = 128 * dti
                    ps = psPX.tile([128, QCH], F32, tag="psPX")
                    for kti in range(KT):
                        nc.tensor.matmul(
                            ps[:],
                            xr_t[kti // 4][:, kti % 4, d0 : d0 + 128],
                            p_ap[c][:, kti, :],
                            start=(kti == 0),
                            stop=(kti == KT - 1),
                        )
                    nc.vector.tensor_copy(pxt[:, dti, :], ps[:])
                    if piggy:
                        for _ in range(min(2, len(piggy))):
                            piggy.pop(0)()
                while piggy:
                    piggy.pop(0)()
                for et in range(ET):
                    ops = psPO.tile([128, QCH], F32, tag="psPO")
                    for dti in range(DT):
                        nc.tensor.matmul(
                            ops[:],
                            wv_t[et][:, dti, :],
                            pxt[:, dti, :],
                            start=(dti == 0),
                            stop=(dti == DT - 1),
                        )
                    osb = outpool.tile([128, QCH], F16, tag="osb")
                    nc.vector.scalar_tensor_tensor(
                        osb[:], ops[:], 1.0, recb[:], op0=AL.mult, op1=AL.mult
                    )
                    nc.sync.dma_start(ot_r[:, et, q0 : q0 + QCH], osb[:])

            def stt_exp_ops(c):
                """Interleaved stt + exp thunks for chunk c (exp batch after
                each 4th stt); allocates p(c) eagerly."""
                st, maxb = st_ap[c], maxb_ap[c]
                p_ap[c] = ppool.tile([128, KT, QCH], F16, tag="p", name="p")
                ops = []
                for kti in range(KT):
                    def stt(k=kti):
                        nc.vector.scalar_tensor_tensor(
                            st[:, k, :], st[:, k, :], 10.0, maxb[:],
                            op0=AL.mult, op1=AL.subtract,
                        )
                    ops.append(stt)
                    if kti % 4 == 3:
                        def expb(k=kti, c=c):
                            nc.scalar.activation(
                                p_ap[c][:, k - 3 : k + 1, :],
                                st[:, k - 3 : k + 1, :],
                                EXP,
                            )
                        ops.append(expb)
                return ops

            emit_qk_half(0, 0)
            emit_qk_half(0, 1)
            emit_qk_half(1, 0)
            emit_tree_finish(0)
            emit_maxb(0)
            emit_qk_half(1, 1, piggy=stt_exp_ops(0))
            recb0 = emit_sum_recb(0)
            # chunk 1's softmax prep is hoisted before chunk 0's PX/PO so
            # it runs underneath PX(0)/PO(0); exp(1) is emitted after PX(0)
            # because the single P buffer imposes a write-after-read there
            emit_tree_finish(1)
            emit_maxb(1)
            st1, maxb1 = st_ap[1], maxb_ap[1]
            stt1 = [
                (lambda k=k: nc.vector.scalar_tensor_tensor(
                    st1[:, k, :], st1[:, k, :], 10.0, maxb1[:],
                    op0=AL.mult, op1=AL.subtract,
                ))
                for k in range(KT)
            ]
            emit_px_po(0, recb0, piggy=stt1)
            emit_exp(1)
            recb1 = emit_sum_recb(1)
            emit_px_po(1, recb1)

    nc.compile()
    _BUILT["nc"] = nc
    return nc


def _tile_cols(a, blocks):
    """[D_in, cols] -> [blocks, 128, D_in//128, cols/blocks]: row d of
    block j lands at [j, d % 128, d // 128, :]."""
    cols = a.shape[1] // blocks
    t = a.reshape(-1, 128, a.shape[1]).transpose(1, 0, 2)
    out = np.empty((blocks, 128, t.shape[1], cols), a.dtype)
    for j in range(blocks):
        out[j] = t[:, :, j * cols : (j + 1) * cols]
    return np.ascontiguousarray(out)


def _tile_rowblocks(a, blocks):
    """[rows, cols] -> [blocks, 128, rows//128//blocks, cols]: row r lands
    at [rt // (rows//128//blocks), r % 128, rt % ..., :] with rt = r//128."""
    t = a.reshape(-1, 128, a.shape[1]).transpose(1, 0, 2)  # [128, RT, cols]
    rt = t.shape[1] // blocks
    out = np.empty((blocks, 128, rt, a.shape[1]), a.dtype)
    for j in range(blocks):
        out[j] = t[:, j * rt : (j + 1) * rt, :]
    return np.ascontiguousarray(out)


def _prep_inputs(x, q_w, k_w, v_w):
    a_full = (q_w.T.astype(np.float64) @ k_w.astype(np.float64)).astype(
        np.float32
    )
    am = _tile_cols(a_full.astype(np.float16), ET)
    wv = _tile_cols(v_w.T.astype(np.float16), ET)

    xk_b, xr_b = [], []
    for b in range(B):
        xb = np.asarray(x[b]).astype(np.float16)
        xk_b.append(_tile_cols(np.ascontiguousarray(xb.T), 4))
        xr_b.append(_tile_rowblocks(xb, 4))

    in_maps = []
    for core in range(NCORES):
        b, h = divmod(core, 2)
        xq2 = _tile_cols(
            np.ascontiguousarray(np.asarray(x[b, NQ * h : NQ * (h + 1)]).T).astype(
                np.float16
            ),
            2,
        )
        # re-block [2, 128, 8, 512] -> [8, 128, 2, 512] (dti pairs)
        xq = np.ascontiguousarray(
            xq2.reshape(2, 128, 4, 2, 512).transpose(0, 2, 1, 3, 4).reshape(
                8, 128, 2, 512
            )
        )
        in_maps.append(
            {"xq": xq, "xk": xk_b[b], "xr": xr_b[b], "am": am, "wv": wv}
        )
    return in_maps


def run(x, q_w, k_w, v_w, trace=False):
    from concourse.bass_utils import run_bass_kernel_spmd

    nc = _build()
    in_maps = _prep_inputs(x, q_w, k_w, v_w)
    res = run_bass_kernel_spmd(nc, in_maps, list(range(NCORES)), trace=trace)
    out = np.empty((B, SEQ, D), np.float32)
    for core in range(NCORES):
        b, h = divmod(core, 2)
        out[b, NQ * h : NQ * (h + 1)] = res.results[core]["ot"].T.astype(np.float32)
    return out, res


def kernel(x, q_w, k_w, v_w):
    x = np.asarray(x, np.float32)
    q_w = np.asarray(q_w, np.float32)
    k_w = np.asarray(k_w, np.float32)
    v_w = np.asarray(v_w, np.float32)
    out, _ = run(x, q_w, k_w, v_w, trace=False)
    return out
